# revision 38
# baseline (speedup 1.0000x reference)
"""Trainium2 Bass kernel for nn_DecoderSmoothedMaxPoolingLoss.

Loss (see reference):
  neg  = -log(1 - X)                                    (B,T,K)
  loss = sum_{b, t<len_b, k} neg
         - sum_{b, i in [0,Lw_b), k=tgt_b} neg[b, tau_s_b + i, k]
         + sum_b -log( max_j  clip(conv_same(win_b * valid_b, filt), EPS, 1) * valid_b )
  where tau_s = max(0, w_end + 40 - 60), tau_e = min(tau_s + 60, len),
  Lw = tau_e - tau_s, win_b[i] = X[b, tau_s_b + i, tgt_b].

Sharding: pure data parallel over batch — 8 batches per core on 8 cores.
Each core computes its partial scalar loss on device; host sums the 8
partials (the "all-reduce").

Key numeric transform: the host ships Xn = fp8_e5m2(1 - X) with the
invalid tail (t >= len_b) set to 1.0.  ln() only cares about the
RELATIVE error of (1 - X), which e5m2 bounds at 2^-3 uniformly ((1-X)
is in [1e-4, 1], all e5m2-normal), so the summed loss error is ~2.6e-3
(tolerance 2e-2) while HBM traffic QUARTERS (3.2 MB/core).  Invalid
positions contribute ln(1) = 0, so no mask is needed anywhere.

Per core (viewed flat as (128, 25000) fp8):
  big term:  6 chunks (128, F) on the sync HWDGE ring (small first for
             pipeline warm-up, tiny last chunk skips the fold).
             Per chunk: one DVE tensor_tensor multiplying the two
             contiguous halves (fp8 in, bf16 out) -> products
             (128, F/2) [ln(a*b) = ln a + ln b], then ONE ACT
             instruction: Ln with fused accum_out -> column of C.
  windows:   the host extracts win values X[b, tau_s+i, tgt] exactly
             (an index gather, 480 floats) into aux; the device does
             all window math: exclusion ln-sum, conv as two small
             matmuls, clip/mask/max, pos ln.
  final:     all partial columns live in C (128, NCOL) with a host
             +-1 weight row fixing signs; one matmul with a ones
             vector -> (1, NCOL) PSUM, weight-multiply + reduce ->
             scalar, DMA out.
"""

import numpy as np
import ml_dtypes

import concourse.bass as bass
import concourse.tile as tile
from concourse import bacc
from concourse import mybir
from concourse import bass_utils

AF = mybir.ActivationFunctionType
ALU = mybir.AluOpType
AX = mybir.AxisListType
FP = mybir.dt.float32
BF = mybir.dt.bfloat16
F8 = mybir.dt.float8e5
I32 = mybir.dt.int32

B, T, K = 64, 4000, 100
WIN, OFFSET_D, TRUNC, SIGMA = 60, 40, 21, 9
EPS = 1e-8
NCORES = 8
BLOC = B // NCORES          # 8 batches per core
P = 128                     # SBUF partitions
FTOT = BLOC * T * K // P    # 25000 fp8 per partition
FCH = [4000, 4000, 8000, 6496, 2000, 504]   # all divisible by 4
NCH = len(FCH)
assert sum(FCH) == FTOT
NCOL = NCH + 2              # C columns: chunk sums | excl | pos
# aux cols: M | valid8 | I8 | wrow | winN (host-extracted 1-win values)
AUXW = 2 * WIN + BLOC + NCOL + WIN


def _filt_np():
    half = TRUNC // 2
    x = np.arange(-half, half + 1, dtype=np.float32)
    g = np.exp(-0.5 * (x / SIGMA) ** 2).astype(np.float32)
    g = g / g.sum()
    f = np.zeros(WIN, np.float32)
    c = WIN // 2
    f[c - half:c + half + 1] = g
    return f


def _conv_matrix():
    # smoothed[j] = sum_i win[i] * filt[i - j + pl], pl = (WIN-1)//2
    f = _filt_np()
    pl = (WIN - 1) // 2
    idx = np.arange(WIN)
    u = idx[:, None] - idx[None, :] + pl          # (i, j)
    M = np.where((u >= 0) & (u < WIN), f[np.clip(u, 0, WIN - 1)], 0.0)
    return M.astype(np.float32)


_NC_CACHE = None


def _build_program():
    global _NC_CACHE
    if _NC_CACHE is not None:
        return _NC_CACHE

    nc = bacc.Bacc("TRN2", debug=False)
    Xs = nc.dram_tensor("Xs", [P, FTOT], F8, kind="ExternalInput").ap()
    aux = nc.dram_tensor("aux", [WIN, AUXW], FP, kind="ExternalInput").ap()
    outd = nc.dram_tensor("out", [1, 1], FP, kind="ExternalOutput").ap()

    with tile.TileContext(nc) as tc:
        with tc.tile_pool(name="xin", bufs=1) as xin_pool, \
             tc.tile_pool(name="small", bufs=1) as small, \
             tc.tile_pool(name="psum", bufs=1, space="PSUM") as psum:

            # ---- aux load first on the sync ring (tiny, lane 0) ----
            aux_sb = small.tile([WIN, AUXW], FP)
            nc.sync.dma_start(out=aux_sb[:], in_=aux)

            M_sl = aux_sb[0:WIN, 0:WIN]
            valid_sl = aux_sb[0:BLOC, WIN:2 * WIN]
            I8_sl = aux_sb[0:BLOC, 2 * WIN:2 * WIN + BLOC]
            wrow_sl = aux_sb[0:1, 2 * WIN + BLOC:2 * WIN + BLOC + NCOL]
            winN_sl = aux_sb[0:BLOC, 2 * WIN + BLOC + NCOL:AUXW]

            # ---- bulk chunk loads alternate between the two HWDGE
            # rings (sync=qSPDynamicHW, scalar=qActDynamicHW) so two
            # descriptor generators feed the SDMA engines during the
            # ramp.  The scalar-ring dispatches are emitted before any
            # ACTIVATE, so they are never queued behind ACT work. ----
            xtiles = []
            base = 0
            for ci, F in enumerate(FCH):
                xb = xin_pool.tile([P, F], F8, tag=f"xb{ci}",
                                   name=f"xb{ci}")
                eng = nc.sync if ci % 2 == 0 else nc.scalar
                eng.dma_start(out=xb[:], in_=Xs[:, base:base + F])
                xtiles.append(xb)
                base += F

            C = small.tile([P, NCOL], FP)
            nc.vector.memset(C[:], 0.0)
            ones = small.tile([P, 1], FP)
            nc.vector.memset(ones[:], 1.0)

            # prefetch the Ln table set with a dependency-free dummy ACT
            dummy = small.tile([1, 1], FP)
            nc.scalar.activation(out=dummy[:], in_=ones[0:1, 0:1],
                                 func=AF.Ln)

            # ---- big term: pair-fold product (fp8 -> bf16) then
            # Ln with fused per-partition accumulate on ACT.  The last
            # (tiny) chunk skips the fold so the tail chain is shorter.
            for ci, F in enumerate(FCH):
                xb = xtiles[ci]
                if ci == NCH - 1:
                    nc.scalar.activation(out=xtiles[ci][:], in_=xb[:],
                                         func=AF.Ln,
                                         accum_out=C[0:P, ci:ci + 1])
                    continue
                H = F // 2
                xp = xin_pool.tile([P, H], BF, tag=f"xp{ci}",
                                   name=f"xp{ci}")
                nc.vector.tensor_tensor(out=xp[:], in0=xb[:, 0:H],
                                        in1=xb[:, H:F], op=ALU.mult)
                nc.scalar.activation(out=xp[:], in_=xp[:], func=AF.Ln,
                                     accum_out=C[0:P, ci:ci + 1])
                if ci == 2:
                    # window path from host-extracted winN = 1 - win
                    # win_x = 1 - winN  (= original X at target)
                    win_x = small.tile([BLOC, WIN], FP)
                    nc.vector.tensor_scalar(out=win_x[:], in0=winN_sl,
                                            scalar1=-1.0, scalar2=1.0,
                                            op0=ALU.mult, op1=ALU.add)
                    # exclusion: ln(winN) * valid, row-sum
                    lnw = small.tile([BLOC, WIN], FP)
                    nc.scalar.activation(out=lnw[:], in_=winN_sl,
                                         func=AF.Ln)
                    lnwv = small.tile([BLOC, WIN], FP)
                    nc.vector.tensor_tensor(out=lnwv[:], in0=lnw[:],
                                            in1=valid_sl, op=ALU.mult)
                    nc.vector.tensor_reduce(out=C[0:BLOC, NCH:NCH + 1],
                                            in_=lnwv[:], axis=AX.X,
                                            op=ALU.add)
                    # winv = win_x * valid
                    winv = small.tile([BLOC, WIN], FP)
                    nc.vector.tensor_tensor(out=winv[:], in0=win_x[:],
                                            in1=valid_sl, op=ALU.mult)
                    # conv: transpose winv via matmul with I8, then @ M
                    wvt_ps = psum.tile([WIN, BLOC], FP)
                    nc.tensor.matmul(out=wvt_ps[:], lhsT=winv[:],
                                     rhs=I8_sl, start=True, stop=True)
                    wvt = small.tile([WIN, BLOC], FP)
                    nc.vector.tensor_copy(out=wvt[:], in_=wvt_ps[:])
                    sm_ps = psum.tile([BLOC, WIN], FP)
                    nc.tensor.matmul(out=sm_ps[:], lhsT=wvt[:], rhs=M_sl,
                                     start=True, stop=True)
                    # clip to [EPS, 1]
                    smc = small.tile([BLOC, WIN], FP)
                    nc.vector.tensor_scalar(out=smc[:], in0=sm_ps[:],
                                            scalar1=EPS, scalar2=1.0,
                                            op0=ALU.max, op1=ALU.min)
                    # mask + row max
                    smv = small.tile([BLOC, WIN], FP)
                    nc.vector.tensor_tensor(out=smv[:], in0=smc[:],
                                            in1=valid_sl, op=ALU.mult)
                    mx = small.tile([BLOC, 1], FP)
                    nc.vector.tensor_reduce(out=mx[:], in_=smv[:],
                                            axis=AX.X, op=ALU.max)

            # pos col: ln(mx) per batch
            nc.scalar.activation(out=C[0:BLOC, NCH + 1:NCH + 2], in_=mx[:],
                                 func=AF.Ln)

            # ---- final: tot = sum over columns of wrow * colsum ----
            tot_ps = psum.tile([1, NCOL], FP)
            nc.tensor.matmul(out=tot_ps[:], lhsT=ones[:], rhs=C[:],
                             start=True, stop=True)
            negrow = small.tile([1, NCOL], FP)
            nc.vector.tensor_tensor(out=negrow[:], in0=tot_ps[:],
                                    in1=wrow_sl, op=ALU.mult)
            tot = small.tile([1, 1], FP)
            nc.vector.tensor_reduce(out=tot[:], in_=negrow[:], axis=AX.X,
                                    op=ALU.add)
            nc.sync.dma_start(out=outd, in_=tot[:])

    nc.compile()
    _NC_CACHE = nc
    return nc


def _make_in_maps(X, lengths, tgt, w_end):
    X = np.asarray(X, dtype=np.float32)
    lengths = np.asarray(lengths, dtype=np.int64)
    tgt = np.asarray(tgt, dtype=np.int64)
    w_end = np.asarray(w_end, dtype=np.int64)

    tau_s = np.maximum(0, w_end + OFFSET_D - WIN)
    tau_e = np.minimum(tau_s + WIN, lengths)
    Lw = tau_e - tau_s

    Mmat = _conv_matrix()

    # final-combine weights: big cols and pos get -1, excl gets +1
    # (C holds +sum ln everywhere; loss = -A + Ex - L)
    wrow = np.full(NCOL, -1.0, np.float32)
    wrow[NCH] = 1.0

    in_maps = []
    for cr in range(NCORES):
        bs = slice(cr * BLOC, (cr + 1) * BLOC)
        ls, ts, lw, tg = lengths[bs], tau_s[bs], Lw[bs], tgt[bs]

        # per-core Xn = fp8_e5m2(1 - X), invalid tail -> 1.0 (ln(1)=0)
        Xn = (1.0 - X[bs]).astype(ml_dtypes.float8_e5m2)   # (8, T, K)
        one8 = np.array(1.0, ml_dtypes.float8_e5m2)
        for b in range(BLOC):
            lb = int(ls[b])
            if lb < T:
                Xn[b, lb:] = one8

        # host-extracted window values (exact fp32): 1 - X[b, ts+i, tgt]
        idx_i = ts[:, None] + np.arange(WIN)[None, :]      # (8, WIN)
        winN = 1.0 - X[bs][np.arange(BLOC)[:, None], idx_i, tg[:, None]]

        valid8 = (np.arange(WIN)[None, :] < lw[:, None]).astype(np.float32)
        aux = np.zeros((WIN, AUXW), np.float32)
        aux[0:WIN, 0:WIN] = Mmat
        aux[0:BLOC, WIN:2 * WIN] = valid8
        aux[0:BLOC, 2 * WIN:2 * WIN + BLOC] = np.eye(BLOC, dtype=np.float32)
        aux[0, 2 * WIN + BLOC:2 * WIN + BLOC + NCOL] = wrow
        aux[0:BLOC, 2 * WIN + BLOC + NCOL:AUXW] = winN.astype(np.float32)

        in_maps.append({
            "Xs": Xn.reshape(P, FTOT),
            "aux": aux,
        })
    return in_maps


def kernel(X, lengths, tgt, w_end):
    nc = _build_program()
    in_maps = _make_in_maps(X, lengths, tgt, w_end)
    res = bass_utils.run_bass_kernel_spmd(
        nc, in_maps, core_ids=list(range(NCORES)))
    total = np.float32(0.0)
    for c in range(NCORES):
        total += np.float32(res.results[c]["out"][0, 0])
    return np.array(total, dtype=np.float32)


# revision 43
# speedup vs baseline: 1.0181x; 1.0181x over previous
"""Trainium2 Bass kernel for nn_DecoderSmoothedMaxPoolingLoss.

Loss (see reference):
  neg  = -log(1 - X)                                    (B,T,K)
  loss = sum_{b, t<len_b, k} neg
         - sum_{b, i in [0,Lw_b), k=tgt_b} neg[b, tau_s_b + i, k]
         + sum_b -log( max_j  clip(conv_same(win_b * valid_b, filt), EPS, 1) * valid_b )
  where tau_s = max(0, w_end + 40 - 60), tau_e = min(tau_s + 60, len),
  Lw = tau_e - tau_s, win_b[i] = X[b, tau_s_b + i, tgt_b].

Sharding: pure data parallel over batch — 8 batches per core on 8 cores.
Each core computes its partial scalar loss on device; host sums the 8
partials (the "all-reduce").

Key numeric transform: the host ships Xn = fp8_e5m2(1 - X) with the
invalid tail (t >= len_b) set to 1.0.  ln() only cares about the
RELATIVE error of (1 - X), which e5m2 bounds at 2^-3 uniformly ((1-X)
is in [1e-4, 1], all e5m2-normal), so the summed loss error is ~2.6e-3
(tolerance 2e-2) while HBM traffic QUARTERS (3.2 MB/core).  Invalid
positions contribute ln(1) = 0, so no mask is needed anywhere.

Per core (viewed flat as (128, 25000) fp8):
  big term:  5 descending-size chunks (128, F) on the sync HWDGE ring.
             Per chunk: one DVE tensor_tensor multiplying the two
             contiguous halves (fp8 in, bf16 out) -> products
             (128, F/2) [ln(a*b) = ln a + ln b], then ONE ACT
             instruction: Ln with fused accum_out -> column of C.
  windows:   the host extracts win values X[b, tau_s+i, tgt] exactly
             (an index gather, 480 floats) into aux; the device does
             all window math: exclusion ln-sum, conv as two small
             matmuls, clip/mask/max, pos ln.
  final:     all partial columns live in C (128, NCOL) with a host
             +-1 weight row fixing signs; one matmul with a ones
             vector -> (1, NCOL) PSUM, weight-multiply + reduce ->
             scalar, DMA out.
"""

import numpy as np
import ml_dtypes

import concourse.bass as bass
import concourse.tile as tile
from concourse import bacc
from concourse import mybir
from concourse import bass_utils

AF = mybir.ActivationFunctionType
ALU = mybir.AluOpType
AX = mybir.AxisListType
FP = mybir.dt.float32
BF = mybir.dt.bfloat16
F8 = mybir.dt.float8e5
I32 = mybir.dt.int32

B, T, K = 64, 4000, 100
WIN, OFFSET_D, TRUNC, SIGMA = 60, 40, 21, 9
EPS = 1e-8
NCORES = 8
BLOC = B // NCORES          # 8 batches per core
P = 128                     # SBUF partitions
FTOT = BLOC * T * K // P    # 25000 fp8 per partition
FCH = [1000, 3000, 8000, 6496, 4000, 2000, 504]   # warm-up head, small tail
NCH = len(FCH)
assert sum(FCH) == FTOT
NCOL = NCH + 2              # C columns: chunk sums | excl | pos
# aux cols: M | valid8 | I8 | wrow | winN (host-extracted 1-win values)
AUXW = 2 * WIN + BLOC + NCOL + WIN


def _filt_np():
    half = TRUNC // 2
    x = np.arange(-half, half + 1, dtype=np.float32)
    g = np.exp(-0.5 * (x / SIGMA) ** 2).astype(np.float32)
    g = g / g.sum()
    f = np.zeros(WIN, np.float32)
    c = WIN // 2
    f[c - half:c + half + 1] = g
    return f


def _conv_matrix():
    # smoothed[j] = sum_i win[i] * filt[i - j + pl], pl = (WIN-1)//2
    f = _filt_np()
    pl = (WIN - 1) // 2
    idx = np.arange(WIN)
    u = idx[:, None] - idx[None, :] + pl          # (i, j)
    M = np.where((u >= 0) & (u < WIN), f[np.clip(u, 0, WIN - 1)], 0.0)
    return M.astype(np.float32)


_NC_CACHE = None


def _build_program():
    global _NC_CACHE
    if _NC_CACHE is not None:
        return _NC_CACHE

    nc = bacc.Bacc("TRN2", debug=False)
    Xs = nc.dram_tensor("Xs", [P, FTOT], F8, kind="ExternalInput").ap()
    aux = nc.dram_tensor("aux", [WIN, AUXW], FP, kind="ExternalInput").ap()
    outd = nc.dram_tensor("out", [1, 1], FP, kind="ExternalOutput").ap()

    with tile.TileContext(nc) as tc:
        with tc.tile_pool(name="xin", bufs=1) as xin_pool, \
             tc.tile_pool(name="small", bufs=1) as small, \
             tc.tile_pool(name="psum", bufs=1, space="PSUM") as psum:

            # ---- aux load first on the sync ring (tiny, lane 0) ----
            aux_sb = small.tile([WIN, AUXW], FP)
            nc.sync.dma_start(out=aux_sb[:], in_=aux)

            M_sl = aux_sb[0:WIN, 0:WIN]
            valid_sl = aux_sb[0:BLOC, WIN:2 * WIN]
            I8_sl = aux_sb[0:BLOC, 2 * WIN:2 * WIN + BLOC]
            wrow_sl = aux_sb[0:1, 2 * WIN + BLOC:2 * WIN + BLOC + NCOL]
            winN_sl = aux_sb[0:BLOC, 2 * WIN + BLOC + NCOL:AUXW]

            # ---- bulk chunk loads on the sync HWDGE ring ----
            xtiles = []
            base = 0
            for ci, F in enumerate(FCH):
                xb = xin_pool.tile([P, F], F8, tag=f"xb{ci}",
                                   name=f"xb{ci}")
                nc.sync.dma_start(out=xb[:], in_=Xs[:, base:base + F])
                xtiles.append(xb)
                base += F

            C = small.tile([P, NCOL], FP)
            nc.vector.memset(C[:], 0.0)
            ones = small.tile([P, 1], FP)
            nc.vector.memset(ones[:], 1.0)

            # prefetch the Ln table set with a dependency-free dummy ACT
            dummy = small.tile([1, 1], FP)
            nc.scalar.activation(out=dummy[:], in_=ones[0:1, 0:1],
                                 func=AF.Ln)

            # ---- big term: pair-fold product (fp8 -> bf16) then
            # Ln with fused per-partition accumulate on ACT.  The last
            # (tiny) chunk skips the fold so the tail chain is shorter.
            for ci, F in enumerate(FCH):
                xb = xtiles[ci]
                if ci == NCH - 1:
                    nc.scalar.activation(out=xtiles[ci][:], in_=xb[:],
                                         func=AF.Ln,
                                         accum_out=C[0:P, ci:ci + 1])
                    continue
                H = F // 2
                xp = xin_pool.tile([P, H], BF, tag=f"xp{ci}",
                                   name=f"xp{ci}")
                nc.vector.tensor_tensor(out=xp[:], in0=xb[:, 0:H],
                                        in1=xb[:, H:F], op=ALU.mult)
                nc.scalar.activation(out=xp[:], in_=xp[:], func=AF.Ln,
                                     accum_out=C[0:P, ci:ci + 1])
                if ci == 1:
                    # window path from host-extracted winN = 1 - win
                    # win_x = 1 - winN  (= original X at target)
                    win_x = small.tile([BLOC, WIN], FP)
                    nc.vector.tensor_scalar(out=win_x[:], in0=winN_sl,
                                            scalar1=-1.0, scalar2=1.0,
                                            op0=ALU.mult, op1=ALU.add)
                    # exclusion: ln(winN) * valid, row-sum
                    lnw = small.tile([BLOC, WIN], FP)
                    nc.scalar.activation(out=lnw[:], in_=winN_sl,
                                         func=AF.Ln)
                    lnwv = small.tile([BLOC, WIN], FP)
                    nc.vector.tensor_tensor(out=lnwv[:], in0=lnw[:],
                                            in1=valid_sl, op=ALU.mult)
                    nc.vector.tensor_reduce(out=C[0:BLOC, NCH:NCH + 1],
                                            in_=lnwv[:], axis=AX.X,
                                            op=ALU.add)
                    # winv = win_x * valid
                    winv = small.tile([BLOC, WIN], FP)
                    nc.vector.tensor_tensor(out=winv[:], in0=win_x[:],
                                            in1=valid_sl, op=ALU.mult)
                    # conv: transpose winv via matmul with I8, then @ M
                    wvt_ps = psum.tile([WIN, BLOC], FP)
                    nc.tensor.matmul(out=wvt_ps[:], lhsT=winv[:],
                                     rhs=I8_sl, start=True, stop=True)
                    wvt = small.tile([WIN, BLOC], FP)
                    nc.vector.tensor_copy(out=wvt[:], in_=wvt_ps[:])
                    sm_ps = psum.tile([BLOC, WIN], FP)
                    nc.tensor.matmul(out=sm_ps[:], lhsT=wvt[:], rhs=M_sl,
                                     start=True, stop=True)
                    # clip to [EPS, 1]
                    smc = small.tile([BLOC, WIN], FP)
                    nc.vector.tensor_scalar(out=smc[:], in0=sm_ps[:],
                                            scalar1=EPS, scalar2=1.0,
                                            op0=ALU.max, op1=ALU.min)
                    # mask + row max
                    smv = small.tile([BLOC, WIN], FP)
                    nc.vector.tensor_tensor(out=smv[:], in0=smc[:],
                                            in1=valid_sl, op=ALU.mult)
                    mx = small.tile([BLOC, 1], FP)
                    nc.vector.tensor_reduce(out=mx[:], in_=smv[:],
                                            axis=AX.X, op=ALU.max)

            # pos col: ln(mx) per batch
            nc.scalar.activation(out=C[0:BLOC, NCH + 1:NCH + 2], in_=mx[:],
                                 func=AF.Ln)

            # ---- final: tot = sum over columns of wrow * colsum ----
            tot_ps = psum.tile([1, NCOL], FP)
            nc.tensor.matmul(out=tot_ps[:], lhsT=ones[:], rhs=C[:],
                             start=True, stop=True)
            negrow = small.tile([1, NCOL], FP)
            nc.vector.tensor_tensor(out=negrow[:], in0=tot_ps[:],
                                    in1=wrow_sl, op=ALU.mult)
            tot = small.tile([1, 1], FP)
            nc.vector.tensor_reduce(out=tot[:], in_=negrow[:], axis=AX.X,
                                    op=ALU.add)
            nc.sync.dma_start(out=outd, in_=tot[:])

    nc.compile()
    _NC_CACHE = nc
    return nc


def _make_in_maps(X, lengths, tgt, w_end):
    X = np.asarray(X, dtype=np.float32)
    lengths = np.asarray(lengths, dtype=np.int64)
    tgt = np.asarray(tgt, dtype=np.int64)
    w_end = np.asarray(w_end, dtype=np.int64)

    tau_s = np.maximum(0, w_end + OFFSET_D - WIN)
    tau_e = np.minimum(tau_s + WIN, lengths)
    Lw = tau_e - tau_s

    Mmat = _conv_matrix()

    # final-combine weights: big cols and pos get -1, excl gets +1
    # (C holds +sum ln everywhere; loss = -A + Ex - L)
    wrow = np.full(NCOL, -1.0, np.float32)
    wrow[NCH] = 1.0

    in_maps = []
    for cr in range(NCORES):
        bs = slice(cr * BLOC, (cr + 1) * BLOC)
        ls, ts, lw, tg = lengths[bs], tau_s[bs], Lw[bs], tgt[bs]

        # per-core Xn = fp8_e5m2(1 - X), invalid tail -> 1.0 (ln(1)=0)
        Xn = (1.0 - X[bs]).astype(ml_dtypes.float8_e5m2)   # (8, T, K)
        one8 = np.array(1.0, ml_dtypes.float8_e5m2)
        for b in range(BLOC):
            lb = int(ls[b])
            if lb < T:
                Xn[b, lb:] = one8

        # host-extracted window values (exact fp32): 1 - X[b, ts+i, tgt]
        idx_i = ts[:, None] + np.arange(WIN)[None, :]      # (8, WIN)
        winN = 1.0 - X[bs][np.arange(BLOC)[:, None], idx_i, tg[:, None]]

        valid8 = (np.arange(WIN)[None, :] < lw[:, None]).astype(np.float32)
        aux = np.zeros((WIN, AUXW), np.float32)
        aux[0:WIN, 0:WIN] = Mmat
        aux[0:BLOC, WIN:2 * WIN] = valid8
        aux[0:BLOC, 2 * WIN:2 * WIN + BLOC] = np.eye(BLOC, dtype=np.float32)
        aux[0, 2 * WIN + BLOC:2 * WIN + BLOC + NCOL] = wrow
        aux[0:BLOC, 2 * WIN + BLOC + NCOL:AUXW] = winN.astype(np.float32)

        in_maps.append({
            "Xs": Xn.reshape(P, FTOT),
            "aux": aux,
        })
    return in_maps


def kernel(X, lengths, tgt, w_end):
    nc = _build_program()
    in_maps = _make_in_maps(X, lengths, tgt, w_end)
    res = bass_utils.run_bass_kernel_spmd(
        nc, in_maps, core_ids=list(range(NCORES)))
    total = np.float32(0.0)
    for c in range(NCORES):
        total += np.float32(res.results[c]["out"][0, 0])
    return np.array(total, dtype=np.float32)
